# revision 15
# baseline (speedup 1.0000x reference)
"""Trainium2 Bass kernel for a dense transformer decoder layer.

Shapes (hardcoded): B=4, S=2048, D=1024, H=16, HD=64, F=4096, fp32.

Sharding over 8 NeuronCores: core c handles batch b=c//2 and head-half
hh=c%2 (8 of 16 heads, Megatron-style).  Per-head QKV + causal attention
+ the output-projection partial run per core; one ReduceScatter over
core pairs {2b, 2b+1} sums the two head-halves and hands each core its
own 1024-token half; each core then finishes residual + LN1 +
full-weight FFN + LN2 for those tokens.  Host assembles the 8 [D, 1024]
transposed output slices.

Everything on-chip lives in "T-layout" ([feature-on-partitions, tokens])
so no transposes are needed anywhere.  Matmuls run in float32r (~11-bit
mantissa fp32 that streams at bf16 speed); the second FFN matmul runs in
bf16 to halve SBUF for the activation buffer.  LayerNorm stats
(partition-dim sums) and [1,N]->[128,N] broadcasts are ones-matmuls on
the PE.  Softmax runs max-free (scores are O(+-10); exp is safe in
fp32); denominators come from an appended ones-column in V (M=65 pv
matmuls); causality is handled block-wise with 4 host-built diagonal
masks.
"""

import sys

sys.path.insert(0, "/opt/trn_rl_repo")

import numpy as np
import ml_dtypes

import concourse.bass as bass
import concourse.tile as tile
from concourse import bacc, mybir
from concourse.bass import ts, ds
from concourse.bass_utils import run_bass_kernel_spmd

F32 = mybir.dt.float32
F32R = mybir.dt.float32r
BF16 = mybir.dt.bfloat16
AF = mybir.ActivationFunctionType
OP = mybir.AluOpType

B, S, D, H, F = 4, 2048, 1024, 16, 4096
HD = 64
P = 128
KD = D // P  # 8 d-tiles
SB = S // P  # 16 s-blocks
SC = S // 512  # 4 s-chunks (attention)
XC_W = 256  # x streaming chunk width (stage A)
XC_N = S // XC_W
FT = F // P  # 32 f-tiles
TOK = 1024  # tokens owned per core
NC_N = 8
LN_EPS = 1e-5
AR_GROUPS = [[0, 1], [2, 3], [4, 5], [6, 7]]

# ppvec column map (per-partition vectors packed into one [P, 80] tile)
PP_BO, PP_G1, PP_BE1, PP_B2, PP_G2, PP_BE2, PP_B1 = 0, 8, 16, 24, 32, 40, 48


def round_f32r(x: np.ndarray) -> np.ndarray:
    """Round fp32 to the fp32r grid (sign+8exp+11mant in top 20 bits, RNE)."""
    b = np.ascontiguousarray(x, dtype=np.float32).view(np.uint32).astype(np.uint64)
    b = (b + 0x7FF + ((b >> 12) & 1)) & 0xFFFFF000
    return b.astype(np.uint32).view(np.float32)


def build_nc(ar_bypass: bool = False):
    nc = bacc.Bacc("TRN2", target_bir_lowering=False, num_devices=NC_N)

    def din(name, shape, dt=F32R):
        return nc.dram_tensor(name, list(shape), dt, kind="ExternalInput").ap()

    # weight layouts are partition-major on host so every DMA is contiguous
    xT = din("xT", [KD, P, S])  # x[b].T, d-tiled
    x_resid = din("x_resid", [KD, P, TOK], F32)  # exact x slice for residual
    wq = din("wq", [4, P, KD, P])  # [pair, r, d, 2*64], pre-scaled 1/sqrt(HD)
    wk = din("wk", [4, P, KD, P])
    wv = din("wv", [P, KD, 512])
    wo = din("wo", [KD, P, 4, P])  # [m, r, k'-pair, c]
    w1 = din("w1", [FT, P, KD, P], BF16)  # [f, r, d, c]
    w2 = din("w2", [KD, P, FT, P], BF16)  # [m, r, f, c]
    bqk = din("bqk", [P, 8], F32)  # cols 0-3: bq per pair, 4-7: bk per pair
    bv_row = din("bv_row", [1, 512], F32)
    ppvec = din("ppvec", [P, 80], F32)  # bo,g1,be1,b2,g2,be2 (8 each), b1 (32)
    masks = din("masks", [P, 4, 512], BF16)  # causal diag-block masks
    vones = din("vones", [P, SB, 8, 1], BF16)  # ones column for v_aug
    ones2 = din("ones2", [P, 2], F32R)  # LN stats lhsT (M=2)
    ones_row = din("ones_row", [1, P], F32)  # K=1 broadcast lhsT, fp32
    salt = din("salt", [1, 7], F32)  # unique-shape input: avoids stale-cache signature collisions

    out = nc.dram_tensor("out", [KD, P, TOK], F32, kind="ExternalOutput").ap()

    # two ReduceScatters (chunks 0-1, then 2-3) so the first hides under
    # attention compute of the later chunks; each hands the pair-core its
    # own 512-token shard of that half
    ar_in_a = nc.dram_tensor("ar_in_a", [2, D, 512], BF16).ap()
    ar_in_b = nc.dram_tensor("ar_in_b", [2, D, 512], BF16).ap()
    ar_out_a = nc.dram_tensor("ar_out_a", [D, 512], BF16).ap()
    ar_out_b = nc.dram_tensor("ar_out_b", [D, 512], BF16).ap()

    with tile.TileContext(nc) as tc:
        with (
            tc.tile_pool(name="qkv", bufs=1) as qkv_pool,
            tc.tile_pool(name="consts", bufs=1) as consts,
        ):
            # ---- resident constants ----------------------------------------
            mask_sb = consts.tile([P, 4, 512], BF16, name="mask_sb")
            nc.sync.dma_start(out=mask_sb[:], in_=masks[:])
            ones_row_sb = consts.tile([1, P], F32, name="ones_row_sb")
            nc.sync.dma_start(out=ones_row_sb[:], in_=ones_row[:])
            bv_bcast = consts.tile([P, 512], F32, name="bv_bcast")
            nc.sync.dma_start(out=bv_bcast[:], in_=bv_row[:].partition_broadcast(P))
            bqk_sb = consts.tile([P, 8], F32, name="bqk_sb")
            nc.sync.dma_start(out=bqk_sb[:], in_=bqk[:])
            salt_sb = consts.tile([1, 7], F32, name="salt_sb")
            nc.sync.dma_start(out=salt_sb[:], in_=salt[:])

            # ---- stage A: q/k/v projections (x streamed per 256-token chunk)
            qT = [qkv_pool.tile([P, S], F32R, tag=f"qT{p}", name=f"qT{p}") for p in range(4)]
            kT = [qkv_pool.tile([P, S], F32R, tag=f"kT{p}", name=f"kT{p}") for p in range(4)]
            v_one = qkv_pool.tile([P, SB, 8, 65], BF16, name="v_one")
            nc.sync.dma_start(out=v_one[:, :, :, 64:65], in_=vones[:])

            with (
                tc.tile_pool(name="wqk", bufs=1) as wqk_pool,
                tc.tile_pool(name="xchunk", bufs=2) as xch_pool,
                tc.tile_pool(name="ps_qkv", bufs=2, space="PSUM") as ps_qkv,
            ):
                wv_sb = wqk_pool.tile([P, KD, 512], F32R, name="wv_sb")
                nc.sync.dma_start(out=wv_sb[:], in_=wv[:])
                wq_t = [wqk_pool.tile([P, KD, P], F32R, tag=f"wq{p}", name=f"wq{p}") for p in range(4)]
                wk_t = [wqk_pool.tile([P, KD, P], F32R, tag=f"wk{p}", name=f"wk{p}") for p in range(4)]
                for hp in range(4):
                    nc.sync.dma_start(out=wq_t[hp][:], in_=wq[hp])
                    nc.sync.dma_start(out=wk_t[hp][:], in_=wk[hp])
                for n in range(XC_N):
                    xc = xch_pool.tile([P, KD, XC_W], F32R, tag="xc", name="xc")
                    for d in range(KD):
                        nc.sync.dma_start(out=xc[:, d], in_=xT[d][:, ts(n, XC_W)])
                    for hp in range(4):
                        pq = ps_qkv.tile([P, XC_W], F32, tag="pq", name="pq")
                        pk = ps_qkv.tile([P, XC_W], F32, tag="pk", name="pk")
                        for d in range(KD):
                            nc.tensor.matmul(
                                pq[:], lhsT=wq_t[hp][:, d], rhs=xc[:, d],
                                start=(d == 0), stop=(d == KD - 1),
                            )
                        for d in range(KD):
                            nc.tensor.matmul(
                                pk[:], lhsT=wk_t[hp][:, d], rhs=xc[:, d],
                                start=(d == 0), stop=(d == KD - 1),
                            )
                        nc.vector.tensor_scalar_add(
                            qT[hp][:, ts(n, XC_W)], pq[:], bqk_sb[:, hp : hp + 1]
                        )
                        nc.vector.tensor_scalar_add(
                            kT[hp][:, ts(n, XC_W)], pk[:], bqk_sb[:, 4 + hp : 5 + hp]
                        )
                    for sblk in range(XC_W // P):
                        sb = (XC_W // P) * n + sblk
                        pv = ps_qkv.tile([P, 512], F32, tag="pv", name="pv")
                        for d in range(KD):
                            nc.tensor.matmul(
                                pv[:], lhsT=xc[:, d, ts(sblk, P)], rhs=wv_sb[:, d],
                                start=(d == 0), stop=(d == KD - 1),
                            )
                        nc.vector.scalar_tensor_tensor(
                            v_one[:, sb, :, 0:64],
                            pv[:].rearrange("p (h e) -> p h e", h=8),
                            1.0,
                            bv_bcast[:].rearrange("p (h e) -> p h e", h=8),
                            OP.mult,
                            OP.add,
                        )

            # ---- stage B: attention;  stage C: output-projection partial ----
            with (
                tc.tile_pool(name="attn_p", bufs=2) as attn_pool,
                tc.tile_pool(name="probs", bufs=6) as probs_pool,
                tc.tile_pool(name="norm", bufs=2) as norm_pool,
                tc.tile_pool(name="wo_p", bufs=2) as wo_pool,
                tc.tile_pool(name="arbuf", bufs=3) as ar_pool,
                tc.tile_pool(name="ps_sc", bufs=2, space="PSUM") as ps_sc,
                tc.tile_pool(name="ps_at", bufs=2, space="PSUM") as ps_at,
                tc.tile_pool(name="ps_bc", bufs=1, space="PSUM") as ps_bc,
                tc.tile_pool(name="ps_wo", bufs=1, space="PSUM") as ps_wo,
            ):
                for n in range(SC):
                    nblk = 4 * (n + 1)
                    attn_n = attn_pool.tile([P, 4, 512], F32R, tag="attn_n", name="attn_n")
                    for hp in range(4):
                        for e in range(2):
                            h = hp * 2 + e
                            pa = ps_at.tile([65, 512], F32, tag="pa", name="pa")
                            for j2 in range(nblk // 2):
                                j0 = 2 * j2
                                psc = ps_sc.tile([P, 2, 512], F32, tag="psc", name="psc")
                                for dj in range(2):
                                    nc.tensor.matmul(
                                        psc[:, dj],
                                        lhsT=kT[hp][ds(64 * e, 64), ts(j0 + dj, P)],
                                        rhs=qT[hp][ds(64 * e, 64), ts(n, 512)],
                                        start=True, stop=True,
                                    )
                                pr = probs_pool.tile([P, 2, 512], BF16, tag="pr", name="pr")
                                nc.scalar.activation(pr[:], psc[:], AF.Exp)
                                for dj in range(2):
                                    j = j0 + dj
                                    if j // 4 == n:
                                        nc.vector.tensor_tensor(
                                            pr[:, dj], pr[:, dj], mask_sb[:, j % 4], OP.mult
                                        )
                                    nc.tensor.matmul(
                                        pa[:], lhsT=v_one[:, j, h], rhs=pr[:, dj],
                                        start=(j == 0), stop=(j == nblk - 1),
                                    )
                            ssum = norm_pool.tile([1, 512], F32, tag="ssum", name="ssum")
                            nc.scalar.copy(ssum[:], pa[64:65, :])
                            recip = norm_pool.tile([1, 512], F32, tag="recip", name="recip")
                            nc.vector.reciprocal_approx_fast(recip[:], ssum[:])
                            pbc = ps_bc.tile([64, 512], F32, tag="pbc", name="pbc")
                            nc.tensor.matmul(
                                pbc[:], lhsT=ones_row_sb[:, 0:64], rhs=recip[:],
                                start=True, stop=True,
                            )
                            bc_sb = norm_pool.tile([64, 512], F32, tag="bc_sb", name="bc_sb")
                            nc.scalar.copy(bc_sb[:], pbc[:])
                            if e == 0:
                                nc.vector.tensor_tensor(
                                    attn_n[0:64, hp], pa[0:64, :], bc_sb[:], OP.mult
                                )
                            else:
                                tmp = norm_pool.tile([64, 512], F32R, tag="tmp1", name="tmp1")
                                nc.vector.tensor_tensor(
                                    tmp[:], pa[0:64, :], bc_sb[:], OP.mult
                                )
                                nc.sync.dma_start(
                                    out=attn_n[ds(64, 64), hp], in_=tmp[:]
                                )
                    # output projection partial for this token chunk
                    for m in range(KD):
                        wo_t = wo_pool.tile([P, 4, P], F32R, tag="wo", name="wo_t")
                        nc.sync.dma_start(out=wo_t[:], in_=wo[m])
                        pw = ps_wo.tile([P, 512], F32, tag="pw", name="pw")
                        for kp in range(4):
                            nc.tensor.matmul(
                                pw[:], lhsT=wo_t[:, kp], rhs=attn_n[:, kp],
                                start=(kp == 0), stop=(kp == 3),
                            )
                        arb = ar_pool.tile([P, 512], BF16, tag="arb", name="arb")
                        nc.vector.tensor_copy(arb[:], pw[:])
                        ar_tgt = ar_in_a if n < 2 else ar_in_b
                        nc.sync.dma_start(
                            out=ar_tgt[n % 2, ds(m * P, P), :], in_=arb[:]
                        )
                    if n == 1 or n == 3:
                        ar_i, ar_o = (ar_in_a, ar_out_a) if n == 1 else (ar_in_b, ar_out_b)
                        if ar_bypass:
                            nc.sync.dma_start(out=ar_o[:], in_=ar_i[0])
                        else:
                            nc.gpsimd.collective_compute(
                                "ReduceScatter",
                                OP.add,
                                replica_groups=AR_GROUPS,
                                ins=[ar_i[:]],
                                outs=[ar_o[:]],
                            )

        # ---- stage D: residual + LN1;  stage E: FFN;  LN2; output ---------
        with (
            tc.tile_pool(name="post", bufs=1) as post,
            tc.tile_pool(name="consts2", bufs=1) as consts2,
        ):
            ones2_sb2 = consts2.tile([P, 2], F32R, name="ones2_sb2")
            nc.sync.dma_start(out=ones2_sb2[:], in_=ones2[:])
            ones_row_sb2 = consts2.tile([1, P], F32, name="ones_row_sb2")
            nc.sync.dma_start(out=ones_row_sb2[:], in_=ones_row[:])
            eps_t2 = consts2.tile([1, 1], F32, name="eps_t2")
            nc.vector.memset(eps_t2[:], LN_EPS)
            ppv = consts2.tile([P, 80], F32, name="ppv")
            nc.sync.dma_start(out=ppv[:], in_=ppvec[:])

            def pp(base, i):
                return ppv[:, base + i : base + i + 1]

            def layer_norm_T(r_tiles, g_base, be_base, out_tiles, ps_pool, sqp, statp, ln_tag):
                """r_tiles: KD x [P, TOK] (mutated in place); writes out_tiles."""
                psum_s = ps_pool.tile([2, TOK], F32, tag=f"ps_s_{ln_tag}", name="psum_s")
                psum_q = ps_pool.tile([2, TOK], F32, tag=f"ps_q_{ln_tag}", name="psum_q")
                sq_tiles = []
                for m in range(KD):
                    sq = sqp.tile([P, TOK], F32R, tag="sq", name="sq")
                    nc.scalar.activation(sq[:], r_tiles[m][:], AF.Square)
                    sq_tiles.append(sq)
                for half in range(TOK // 512):
                    for m in range(KD):
                        nc.tensor.matmul(
                            psum_s[:, ts(half, 512)], lhsT=ones2_sb2[:],
                            rhs=r_tiles[m][:, ts(half, 512)],
                            start=(m == 0), stop=(m == KD - 1),
                        )
                    for m in range(KD):
                        nc.tensor.matmul(
                            psum_q[:, ts(half, 512)], lhsT=ones2_sb2[:],
                            rhs=sq_tiles[m][:, ts(half, 512)],
                            start=(m == 0), stop=(m == KD - 1),
                        )
                mean = statp.tile([1, TOK], F32, tag="mean", name="mean")
                nc.vector.tensor_scalar_mul(mean[:], psum_s[0:1, :], 1.0 / D)
                work = statp.tile([1, TOK], F32, tag="work", name="work")
                nc.vector.tensor_scalar_mul(work[:], psum_q[0:1, :], 1.0 / D)
                m2 = statp.tile([1, TOK], F32, tag="m2", name="m2")
                nc.vector.tensor_tensor(m2[:], mean[:], mean[:], OP.mult)
                nc.vector.tensor_tensor(work[:], work[:], m2[:], OP.subtract)
                nc.scalar.activation(work[:], work[:], AF.Sqrt, bias=eps_t2[:])
                rstd = statp.tile([1, TOK], F32, tag="rstd", name="rstd")
                nc.vector.reciprocal(rstd[:], work[:])
                pmb = ps_pool.tile([P, TOK], F32, tag=f"pmb_{ln_tag}", name="pmb")
                prb = ps_pool.tile([P, TOK], F32, tag=f"prb_{ln_tag}", name="prb")
                for half in range(TOK // 512):
                    nc.tensor.matmul(
                        pmb[:, ts(half, 512)], lhsT=ones_row_sb2[:],
                        rhs=mean[:, ts(half, 512)], start=True, stop=True,
                    )
                    nc.tensor.matmul(
                        prb[:, ts(half, 512)], lhsT=ones_row_sb2[:],
                        rhs=rstd[:, ts(half, 512)], start=True, stop=True,
                    )
                for m in range(KD):
                    nc.vector.tensor_tensor(
                        r_tiles[m][:], r_tiles[m][:], pmb[:], OP.subtract
                    )
                    nc.vector.tensor_tensor(
                        r_tiles[m][:], r_tiles[m][:], prb[:], OP.mult
                    )
                    nc.vector.scalar_tensor_tensor(
                        out_tiles[m][:],
                        r_tiles[m][:],
                        pp(g_base, m),
                        pp(be_base, m).to_broadcast((P, TOK)),
                        OP.mult,
                        OP.add,
                    )

            r1 = [post.tile([P, TOK], F32R, tag=f"r1_{m}", name=f"r1_{m}") for m in range(KD)]
            h1 = [post.tile([P, TOK], F32R, tag=f"h1_{m}", name=f"h1_{m}") for m in range(KD)]
            with (
                tc.tile_pool(name="ln1_ps", bufs=1, space="PSUM") as ln1_ps,
                tc.tile_pool(name="ln1_sq", bufs=2) as ln1_sq,
                tc.tile_pool(name="ln1_st", bufs=1) as ln1_st,
                tc.tile_pool(name="arload", bufs=3) as arload,
            ):
                for m in range(KD):
                    art = arload.tile([P, TOK], BF16, tag="art", name="art")
                    nc.sync.dma_start(out=art[:, 0:512], in_=ar_out_a[ds(m * P, P), :])
                    nc.sync.dma_start(out=art[:, 512:1024], in_=ar_out_b[ds(m * P, P), :])
                    xr = arload.tile([P, TOK], F32, tag="xr", name="xr")
                    nc.sync.dma_start(out=xr[:], in_=x_resid[m])
                    nc.vector.scalar_tensor_tensor(
                        r1[m][:], art[:], pp(PP_BO, m), xr[:], OP.add, OP.add
                    )
                layer_norm_T(r1, PP_G1, PP_BE1, h1, ln1_ps, ln1_sq, ln1_st, "ln1")

            # FFN (full weights, own 1024 tokens)
            r2 = r1  # reuse r1 tiles as the pre-LN2 residual buffers
            with (
                tc.tile_pool(name="gbuf", bufs=1) as gbuf,
                tc.tile_pool(name="w1s", bufs=3) as w1s,
                tc.tile_pool(name="w2s", bufs=3) as w2s,
                tc.tile_pool(name="ps_ff", bufs=2, space="PSUM") as ps_ff,
            ):
                gT = gbuf.tile([P, FT, TOK], BF16, name="gT")
                h1b = [gbuf.tile([P, TOK], BF16, tag=f"h1b_{d}", name=f"h1b_{d}") for d in range(KD)]
                for d in range(KD):
                    nc.vector.tensor_copy(h1b[d][:], h1[d][:])
                for f in range(FT):
                    w1_t = w1s.tile([P, KD, P], BF16, tag="w1", name="w1_t")
                    nc.sync.dma_start(out=w1_t[:], in_=w1[f])
                    pg = ps_ff.tile([P, TOK], F32, tag="pg", name="pg")
                    for half in range(TOK // 512):
                        for d in range(KD):
                            nc.tensor.matmul(
                                pg[:, ts(half, 512)],
                                lhsT=w1_t[:, d], rhs=h1b[d][:, ts(half, 512)],
                                start=(d == 0), stop=(d == KD - 1),
                            )
                    nc.scalar.activation(gT[:, f], pg[:], AF.Gelu, bias=pp(PP_B1, f))
                for m in range(KD):
                    w2_t = w2s.tile([P, FT, P], BF16, tag="w2", name="w2_t")
                    nc.sync.dma_start(out=w2_t[:], in_=w2[m])
                    p2 = ps_ff.tile([P, TOK], F32, tag="p2", name="p2")
                    for half in range(TOK // 512):
                        for f in range(FT):
                            nc.tensor.matmul(
                                p2[:, ts(half, 512)],
                                lhsT=w2_t[:, f], rhs=gT[:, f, ts(half, 512)],
                                start=(f == 0), stop=(f == FT - 1),
                            )
                    nc.vector.tensor_scalar_add(p2[:], p2[:], pp(PP_B2, m))
                    nc.vector.tensor_tensor(r2[m][:], p2[:], h1[m][:], OP.add)

            out_tiles = [post.tile([P, TOK], F32, tag=f"o_{m}", name=f"o_{m}") for m in range(KD)]
            with (
                tc.tile_pool(name="ln2_ps", bufs=1, space="PSUM") as ln2_ps,
                tc.tile_pool(name="ln2_sq", bufs=2) as ln2_sq,
                tc.tile_pool(name="ln2_st", bufs=1) as ln2_st,
            ):
                layer_norm_T(r2, PP_G2, PP_BE2, out_tiles, ln2_ps, ln2_sq, ln2_st, "ln2")
            for m in range(KD):
                nc.sync.dma_start(out=out[m], in_=out_tiles[m][:])

    nc.compile()
    return nc


def shard_inputs(x, Wq, bq_, Wk, bk_, Wv, bv_, Wo, bo, W1, b1, W2, b2, g1, be1, g2, be2):
    """Build the per-core in_maps (all numpy, host-side)."""
    x = np.asarray(x, np.float32)
    Wq = np.asarray(Wq, np.float32) / np.sqrt(HD)
    Wk = np.asarray(Wk, np.float32)
    Wv = np.asarray(Wv, np.float32)
    Wo = np.asarray(Wo, np.float32)
    W1 = np.asarray(W1, np.float32)
    W2 = np.asarray(W2, np.float32)

    # shared, core-independent tensors
    w1_t = np.ascontiguousarray(
        W1.reshape(KD, P, FT, P).transpose(2, 1, 0, 3)
    ).astype(ml_dtypes.bfloat16)  # w1[f, r, d, c] = W1[d*128+r, f*128+c]
    w2_t = np.ascontiguousarray(
        W2.reshape(FT, P, KD, P).transpose(2, 1, 0, 3)
    ).astype(ml_dtypes.bfloat16)  # w2[m, r, f, c] = W2[f*128+r, m*128+c]

    ppvec = np.zeros((P, 80), np.float32)
    for base, vec in [
        (PP_BO, bo), (PP_G1, g1), (PP_BE1, be1), (PP_B2, b2), (PP_G2, g2), (PP_BE2, be2),
    ]:
        ppvec[:, base : base + KD] = np.asarray(vec, np.float32).reshape(KD, P).T
    ppvec[:, PP_B1 : PP_B1 + FT] = np.asarray(b1, np.float32).reshape(FT, P).T

    iota = np.arange(512)
    masks = np.zeros((4, P, 512), np.float32)
    for jj in range(4):
        masks[jj] = (iota[None, :] >= (P * jj + np.arange(P))[:, None]).astype(np.float32)
    masks = np.ascontiguousarray(masks.transpose(1, 0, 2))  # [P, 4, 512]
    vones = np.ones((P, SB, 8, 1), ml_dtypes.bfloat16)
    ones2 = np.ones((P, 2), np.float32)
    ones_row = np.ones((1, P), np.float32)

    in_maps = []
    for c in range(NC_N):
        b_i, hh = c // 2, c % 2
        heads = slice(hh * 8, hh * 8 + 8)
        xT_c = round_f32r(x[b_i].T.reshape(KD, P, S))
        own = np.r_[hh * 512 : hh * 512 + 512, 1024 + hh * 512 : 1024 + hh * 512 + 512]
        x_resid_c = np.ascontiguousarray(x[b_i][own].T.reshape(KD, P, TOK))

        Wq8 = Wq[heads].reshape(8, KD, P, HD)  # [h, d, r, e]
        Wk8 = Wk[heads].reshape(8, KD, P, HD)
        Wv8 = Wv[heads]  # [8, D, HD]
        wq_c = np.empty((4, P, KD, P), np.float32)
        wk_c = np.empty((4, P, KD, P), np.float32)
        for p_i in range(4):
            for e in range(2):
                h = 2 * p_i + e
                wq_c[p_i, :, :, e * 64 : (e + 1) * 64] = Wq8[h].transpose(1, 0, 2)
                wk_c[p_i, :, :, e * 64 : (e + 1) * 64] = Wk8[h].transpose(1, 0, 2)
        wv_c = np.ascontiguousarray(
            Wv8.reshape(8, KD, P, HD).transpose(2, 1, 0, 3).reshape(P, KD, 8 * HD)
        )  # wv[r, d, h*64+e] = Wv8[h, d*128+r, e]
        Wo_own = Wo[hh * 512 : (hh + 1) * 512]  # [512, D]
        wo_c = np.ascontiguousarray(
            Wo_own.reshape(4, P, KD, P).transpose(2, 1, 0, 3)
        )  # wo[m, r, kp, c] = Wo_own[kp*128+r, m*128+c]

        bq8 = np.asarray(bq_, np.float32)[heads].reshape(4, P)
        bk8 = np.asarray(bk_, np.float32)[heads].reshape(4, P)
        bqk_c = np.concatenate([bq8.T, bk8.T], axis=1)  # [P, 8]
        bv8 = np.asarray(bv_, np.float32)[heads]

        in_maps.append(
            {
                "xT": xT_c,
                "x_resid": x_resid_c,
                "wq": round_f32r(wq_c),
                "wk": round_f32r(wk_c),
                "wv": round_f32r(wv_c),
                "wo": round_f32r(wo_c),
                "w1": w1_t,
                "w2": w2_t,
                "bqk": bqk_c,
                "bv_row": bv8.reshape(1, 8 * HD),
                "ppvec": ppvec,
                "masks": masks.astype(ml_dtypes.bfloat16),
                "vones": vones,
                "ones2": ones2,
                "ones_row": ones_row,
                "salt": np.full((1, 7), 7.0, np.float32),
            }
        )
    return in_maps


_NC_CACHE = {}


def _get_nc(ar_bypass=False):
    key = bool(ar_bypass)
    if key not in _NC_CACHE:
        _NC_CACHE[key] = build_nc(ar_bypass)
    return _NC_CACHE[key]


def assemble(results):
    out = np.empty((B, S, D), np.float32)
    for c in range(NC_N):
        b_i, hh = c // 2, c % 2
        own = np.r_[hh * 512 : hh * 512 + 512, 1024 + hh * 512 : 1024 + hh * 512 + 512]
        oT = results[c]["out"].reshape(D, TOK)
        out[b_i, own, :] = oT.T
    return out


def kernel(**inputs) -> np.ndarray:
    nc = _get_nc()
    in_maps = shard_inputs(
        inputs["x"], inputs["Wq"], inputs["bq"], inputs["Wk"], inputs["bk"],
        inputs["Wv"], inputs["bv"], inputs["Wo"], inputs["bo"],
        inputs["W1"], inputs["b1"], inputs["W2"], inputs["b2"],
        inputs["g1"], inputs["be1"], inputs["g2"], inputs["be2"],
    )
    res = run_bass_kernel_spmd(nc, in_maps, list(range(NC_N)))
    return assemble(res.results)
